# revision 56
# baseline (speedup 1.0000x reference)
"""Conformer block kernel for 8 Trainium2 NeuronCores.

Sharding: pure data-parallel over batch (B=8 -> 1 batch element per core,
zero collectives). All weights are replicated; BatchNorm affines and scalar
multipliers are folded into the adjacent pointwise-conv weights on the host.

Matmul precision: everything runs as float32r (full PE rate) EXCEPT the
dense depthwise-ish conv (1024x1024xK=31 contraction, ~85% of all FLOPs),
which runs as fp8(e4m3) with perf_mode=DoubleRow (2 contraction rows per
PE pass). Weights are pre-scaled by 2^6 on the host (and the GLU 'a' path
by 2^2) so fp8 values sit in the normal range; the product scale 2^-8 is
folded into the PSUM->SBUF Silu activation.

Other PE savings vs the plain version: the softmax denominator rides as a
65th "ones" column of V (no separate ones-matmul), and the S=64-contraction
score matmuls for a head pair run concurrently in disjoint PE row groups
via tile_position.
"""
import sys

sys.path.insert(0, '/opt/trn_rl_repo')

import numpy as np
import ml_dtypes
import concourse.bass as bass
import concourse.tile as tile
from concourse import bacc, mybir
from concourse.bass_utils import run_bass_kernel_spmd

F32 = mybir.dt.float32
F32R = mybir.dt.float32r
BF16 = mybir.dt.bfloat16
FP8 = mybir.dt.float8e4
AF = mybir.ActivationFunctionType
ALU = mybir.AluOpType
DR = mybir.MatmulPerfMode.DoubleRow
E4NP = ml_dtypes.float8_e4m3fn

B, DIM, S = 8, 512, 1024
H, DH = 8, 64
FF_INNER = 1024
CONV_INNER = 1024
K = 31
PAD = (K - 1) // 2
N_CORES = 8

CT = DIM // 128          # 4  channel tiles of the 512-dim stream
UT = CONV_INNER // 128   # 8  tiles of 1024-wide inner dims
SC = S // 512            # 2  free-dim chunks of 512
CP = UT // 2             # 4  fp8 pair-tiles of the conv inner dim
UPW = PAD + S + 17       # 1056: padded conv input row (15 left, 17 right)

W_SCALE = 64.0           # dconv weight pre-scale (keeps fp8 out of subnormals)
U_SCALE = 1.0            # GLU 'a'-path pre-scale (no measurable gain beyond 1)
PSUM_SCALE = 1.0 / (W_SCALE * U_SCALE)


def _host_prep(i):
    """Fold affines/scalars into weights; pre-transpose for lhsT layout."""
    f = np.float32
    w = {}

    BF = ml_dtypes.bfloat16

    def fold(wmat, g, b, bout, dt=f):
        # y = wmat @ (g*x + b) + bout  ->  W' = wmat * g[None, :],
        # b' = wmat @ b + bout ; return transposed W' [in, out]
        wp = (wmat * g[None, :]).astype(f)
        bp = (wmat @ b + bout).astype(f)
        return np.ascontiguousarray(wp.T).astype(dt), bp

    w['w_ff1_1'], w['b_ff1_1'] = fold(i['ff1_w1'], i['ff1_g'], i['ff1_b'], i['ff1_b1'], BF)
    w['w_ff1_2'] = np.ascontiguousarray((0.5 * i['ff1_w2']).T.astype(f)).astype(BF)
    w['b_ff1_2'] = (0.5 * i['ff1_b2']).astype(f)

    w['w_q'], w['b_q'] = fold(i['wq'], i['attn_g'], i['attn_b'], i['bq'])
    w['w_k'], w['b_k'] = fold(i['wk'], i['attn_g'], i['attn_b'], i['bk'])
    w['w_v'], w['b_v'] = fold(i['wv'], i['attn_g'], i['attn_b'], i['bv'])
    w['w_o'] = np.ascontiguousarray(i['wo'].T.astype(f))
    w['b_o'] = i['bo'].astype(f)

    # pw1: scale the GLU 'a' half (first CONV_INNER out channels) by U_SCALE
    # so the fp8 conv input u = U_SCALE * a * sigmoid(gate).
    pw1_w = i['pw1_w'].copy()
    pw1_b = i['pw1_b'].copy()
    pw1_w[:CONV_INNER] *= U_SCALE
    pw1_b[:CONV_INNER] *= U_SCALE
    w['w_pw1'], w['b_pw1'] = fold(pw1_w, i['conv_g'], i['conv_b'], pw1_b, BF)

    # dconv: fold cbn_g into weights, pre-scale by W_SCALE, quantize to fp8.
    # Layout per (ot, cp): a [128(c_lo), 31*2*128] lhsT strip where the free
    # offset is k*256 + c_hi*128 + co — each k-slice is a DoubleRow
    # [128, 2, 128] stationary operand pairing channels c_hi=0/1.
    dw = (i['dconv_w'][:, :, 0, :] * i['cbn_g'][:, None, None]).astype(f)  # [o,c,k]
    dw = np.clip(dw * W_SCALE, -240.0, 240.0)
    a = dw.reshape(UT, 128, CP, 2, 128, K)         # [ot, co, cp, chi, clo, k]
    a = a.transpose(0, 2, 4, 5, 3, 1)              # [ot, cp, clo, k, chi, co]
    w['w_dc'] = np.ascontiguousarray(a.reshape(UT, CP, 128, K * 256)).astype(E4NP)
    w['b_dc'] = (i['cbn_g'] * i['dconv_b'] + i['cbn_b']).astype(f)
    w['w_pw2'] = np.ascontiguousarray(i['pw2_w'].T.astype(f)).astype(BF)
    w['b_pw2'] = i['pw2_b'].astype(f)

    w['w_ff2_1'], w['b_ff2_1'] = fold(i['ff2_w1'], i['ff2_g'], i['ff2_b'], i['ff2_b1'], BF)
    w['w_ff2_2'] = np.ascontiguousarray((0.5 * i['ff2_w2']).T.astype(f)).astype(BF)
    w['b_ff2_2'] = (0.5 * i['ff2_b2']).astype(f)

    w['fin_g'] = i['fin_g'].astype(f)
    w['fin_b'] = i['fin_b'].astype(f)
    w['zpad'] = np.zeros(17, E4NP)
    w['onesv'] = np.ones(64, f)
    return w


def _bias_tile(nc, sb, dram_vec, n):
    """Load a [n*128] DRAM vector as a [128, n] SBUF tile (col t = tile t)."""
    t = sb.tile([128, n], F32, tag=f'bias_{dram_vec.name}', name=f'b_{dram_vec.name}')
    nc.sync.dma_start(t[:], dram_vec.ap().rearrange('(t p) -> p t', p=128))
    return t


def _bcast_tile(nc, sb, dram_vec, n, tag):
    """Broadcast a [n] DRAM vector across 128 partitions -> [128, n] f32r."""
    t = sb.tile([128, n], F32R, tag=tag, name=tag)
    v = dram_vec.ap()
    nc.sync.dma_start(
        t[:], bass.AP(tensor=v.tensor, offset=0, ap=[[0, 128], [1, n]]))
    return t


def build_program():
    nc = bacc.Bacc('TRN2', target_bir_lowering=False, debug=False)
    dt_in = {}

    def din(name, shape, dt=F32R):
        dt_in[name] = nc.dram_tensor(name, shape, dt, kind='ExternalInput')
        return dt_in[name]

    din('x', [DIM, S])
    din('w_ff1_1', [DIM, FF_INNER], BF16); din('b_ff1_1', [FF_INNER], F32)
    din('w_ff1_2', [FF_INNER, DIM], BF16); din('b_ff1_2', [DIM], F32)
    din('w_q', [DIM, DIM]); din('b_q', [DIM], F32)
    din('w_k', [DIM, DIM]); din('b_k', [DIM], F32)
    din('w_v', [DIM, DIM]); din('b_v', [DIM])
    din('w_o', [DIM, DIM]); din('b_o', [DIM], F32)
    din('w_pw1', [DIM, 2 * CONV_INNER], BF16); din('b_pw1', [2 * CONV_INNER], F32)
    din('w_dc', [UT, CP, 128, K * 256], FP8)
    din('b_dc', [CONV_INNER], F32)
    din('w_pw2', [CONV_INNER, DIM], BF16); din('b_pw2', [DIM], F32)
    din('w_ff2_1', [DIM, FF_INNER], BF16); din('b_ff2_1', [FF_INNER], F32)
    din('w_ff2_2', [FF_INNER, DIM], BF16); din('b_ff2_2', [DIM], F32)
    din('fin_g', [DIM], F32); din('fin_b', [DIM], F32)
    din('zpad', [17], FP8)
    din('onesv', [64])
    out_d = nc.dram_tensor('out', [DIM, S], F32, kind='ExternalOutput')

    with tile.TileContext(nc, pool_alloc_mode='queue') as tc:
        _emit(nc, tc, dt_in, out_d)
    nc.compile()
    return nc


def _emit(nc, tc, din, out_d):
    from contextlib import ExitStack
    ctx = ExitStack()
    with ctx:
        # ---- persistent pools -------------------------------------------
        resid = ctx.enter_context(tc.tile_pool(name='resid', bufs=2))
        hid = ctx.enter_context(tc.tile_pool(name='hid', bufs=1))
        btp = ctx.enter_context(tc.tile_pool(name='biases', bufs=1))

        def new_resid(i):
            return resid.tile([128, S], F32R, tag=f'r{i}', name=f'r{i}')

        def hid_tile(i):
            return hid.tile([128, S], F32R, tag=f'h{i}', name=f'h{i}')

        def hidb_tile(i):
            return hid.tile([128, S], BF16, tag=f'hb{i}', name=f'hb{i}')

        xcast = ctx.enter_context(tc.tile_pool(name='xcast', bufs=2))

        def cast_bf16(x_in):
            out = []
            for i, t in enumerate(x_in):
                b = xcast.tile([128, S], BF16, tag=f'xb{i}', name=f'xb{i}')
                with nc.allow_low_precision(reason='bf16 matmul input'):
                    nc.vector.tensor_scalar_add(b[:], t[:], 0.0)
                out.append(b)
            return out

        # load x
        x_sb = []
        for i in range(CT):
            t = new_resid(i)
            nc.sync.dma_start(t[:], din['x'].ap()[i * 128:(i + 1) * 128, :])
            x_sb.append(t)

        def load_ff_weights(wp, w1d, w2d):
            w1_sb = []
            for i in range(CT):
                t = wp.tile([128, FF_INNER], BF16, tag=f'w1_{i}')
                nc.sync.dma_start(t[:], w1d.ap()[i * 128:(i + 1) * 128, :])
                w1_sb.append(t)
            w2_sb = []
            for i in range(UT):
                t = wp.tile([128, DIM], BF16, tag=f'w2_{i}')
                nc.sync.dma_start(t[:], w2d.ap()[i * 128:(i + 1) * 128, :])
                w2_sb.append(t)
            return w1_sb, w2_sb

        # Pool alloc order fixes the (LIFO) release order: wqkv is released
        # right after the attention block, wff1 right after FF1. The DMA
        # issue order below is what controls arrival: x, then FF1 weights
        # (the first matmul waits on them), then the attention weights.
        wattn = ctx.enter_context(tc.tile_pool(name='wattn', bufs=1))
        wqkv = tc.alloc_tile_pool(name='wqkv', bufs=1)
        wff1 = tc.alloc_tile_pool(name='wff1', bufs=1)

        ff1_w1_sb, ff1_w2_sb = load_ff_weights(wff1, din['w_ff1_1'],
                                               din['w_ff1_2'])
        wq_sb, wk_sb, wv_sb, wo_sb = [], [], [], []
        for nm, lst in (('w_q', wq_sb), ('w_k', wk_sb), ('w_v', wv_sb),
                        ('w_o', wo_sb)):
            pool = wattn if nm == 'w_o' else wqkv
            for i in range(CT):
                t = pool.tile([128, DIM], F32R, tag=f'{nm}_{i}', name=f'{nm}_{i}')
                nc.sync.dma_start(t[:], din[nm].ap()[i * 128:(i + 1) * 128, :])
                lst.append(t)
        bv_bc = _bcast_tile(nc, wqkv, din['b_v'], DIM, 'bv_bc')
        # ones row AT partition 64 (stationary operand of the reciprocal
        # partition-broadcast matmul, which contracts over array row 64)
        ones_t = wattn.tile([65, 64], F32R, tag='ones_t', name='ones_t')
        ov = din['onesv'].ap()
        nc.sync.dma_start(
            ones_t[64:65, :],
            bass.AP(tensor=ov.tensor, offset=0, ap=[[0, 1], [1, 64]]))
        bq_t = _bias_tile(nc, btp, din['b_q'], CT)
        bk_t = _bias_tile(nc, btp, din['b_k'], CT)
        bo_t = _bias_tile(nc, btp, din['b_o'], CT)

        # ---- generic FF macro -------------------------------------------
        def ff_block(x_in, w1d, b1d, w2d, b2d, nm, preloaded=None):
            with tc.tile_pool(name=f'w{nm}', bufs=1) as wp, \
                 tc.tile_pool(name=f'ps{nm}', bufs=4, space='PSUM') as ps:
                if preloaded is None:
                    w1_sb, w2_sb = load_ff_weights(wp, w1d, w2d)
                else:
                    w1_sb, w2_sb = preloaded
                b1_t = _bias_tile(nc, btp, b1d, UT)
                b2_t = _bias_tile(nc, btp, b2d, CT)

                xb = cast_bf16(x_in)
                h_sb = [hidb_tile(i) for i in range(UT)]
                for ot in range(UT):
                    for sc in range(SC):
                        p = ps.tile([128, 512], F32, tag='pp')
                        for ct in range(CT):
                            nc.tensor.matmul(
                                p[:], w1_sb[ct][:, ot * 128:(ot + 1) * 128],
                                xb[ct][:, sc * 512:(sc + 1) * 512],
                                start=(ct == 0), stop=(ct == CT - 1))
                        with nc.allow_low_precision(reason='ff h bf16'):
                            nc.scalar.activation(
                                h_sb[ot][:, sc * 512:(sc + 1) * 512], p[:],
                                AF.Silu, bias=b1_t[:, ot:ot + 1])
                x_out = []
                for ot in range(CT):
                    t = new_resid(ot)
                    for sc in range(SC):
                        p = ps.tile([128, 512], F32, tag='pp')
                        for ct in range(UT):
                            nc.tensor.matmul(
                                p[:], w2_sb[ct][:, ot * 128:(ot + 1) * 128],
                                h_sb[ct][:, sc * 512:(sc + 1) * 512],
                                start=(ct == 0), stop=(ct == UT - 1))
                        nc.vector.scalar_tensor_tensor(
                            t[:, sc * 512:(sc + 1) * 512], p[:],
                            b2_t[:, ot:ot + 1],
                            x_in[ot][:, sc * 512:(sc + 1) * 512],
                            op0=ALU.add, op1=ALU.add)
                    x_out.append(t)
                return x_out

        # ================= FF1 =================
        x1_sb = ff_block(x_sb, din['w_ff1_1'], din['b_ff1_1'],
                         din['w_ff1_2'], din['b_ff1_2'], 'ff1',
                         preloaded=(ff1_w1_sb, ff1_w2_sb))
        wff1.release()

        # ================= Attention =================
        with tc.tile_pool(name='attn_sb', bufs=1) as asb, \
             tc.tile_pool(name='attn_e', bufs=4) as epool, \
             tc.tile_pool(name='attn_misc', bufs=2) as misc:
            # Q, K projections (standard layout), reusing hid slots
            q_sb = [hid_tile(i) for i in range(CT)]
            k_sb = [hid_tile(CT + i) for i in range(CT)]
            vt_sb = []
            with tc.tile_pool(name='ps_proj1', bufs=2, space='PSUM') as ppp:
                for dst, w_sb, b_t in ((q_sb, wq_sb, bq_t), (k_sb, wk_sb, bk_t)):
                    for ot in range(CT):
                        for sc in range(SC):
                            p = ppp.tile([128, 512], F32, tag='pp')
                            for ct in range(CT):
                                nc.tensor.matmul(
                                    p[:], w_sb[ct][:, ot * 128:(ot + 1) * 128],
                                    x1_sb[ct][:, sc * 512:(sc + 1) * 512],
                                    start=(ct == 0), stop=(ct == CT - 1))
                            nc.scalar.activation(
                                dst[ot][:, sc * 512:(sc + 1) * 512], p[:],
                                AF.Identity, bias=b_t[:, ot:ot + 1])
                # V transposed, with a 65th all-ones column per head so the AV
                # matmul also produces the softmax denominator in row 64.
                # vt[tt] is [128(t), 8*65] : head h at cols 65h..65h+64.
                for tt in range(UT):
                    p = ppp.tile([128, 512], F32, tag='pp')
                    for ct in range(CT):
                        nc.tensor.matmul(
                            p[:], x1_sb[ct][:, tt * 128:(tt + 1) * 128],
                            wv_sb[ct][:], start=(ct == 0), stop=(ct == CT - 1))
                    t = asb.tile([128, 8 * 65], F32R, tag=f'vt{tt}')
                    t3 = t[:].rearrange('p (h d) -> p h d', d=65)
                    nc.vector.tensor_tensor(
                        t3[:, :, 0:64],
                        p[:].rearrange('p (h d) -> p h d', d=64),
                        bv_bc[:].rearrange('p (h d) -> p h d', d=64),
                        op=ALU.add)
                    o8 = din['onesv'].ap()
                    nc.sync.dma_start(
                        t3[:, :, 64:65],
                        bass.AP(tensor=o8.tensor, offset=0,
                                ap=[[0, 128], [1, 8], [0, 1]]))
                    vt_sb.append(t)

            # Per-head attention, software-pipelined over stages (m, sc):
            # stage g issues scores+exp for g interleaved with the AV matmuls
            # of stage g-1, so the PE never idles long enough for the HAM
            # clock gate to re-throttle while the scalar engine chews exps.
            # Head pair (2m, 2m+1) lives at partitions 0-63 / 64-127 of slab
            # m; its two score matmuls run concurrently in disjoint PE row
            # groups via tile_position.
            o_sb = [asb.tile([128, S], F32R, tag=f'o{i}', name=f'o{i}')
                    for i in range(CT)]
            scale = float(DH) ** -0.5

            with tc.tile_pool(name='ps_sc', bufs=2, space='PSUM') as pssc, \
                 tc.tile_pool(name='ps_av', bufs=1, space='PSUM') as psav, \
                 tc.tile_pool(name='ps_rb', bufs=2, space='PSUM') as psrb:

                def finish_heads(m, sc, p_av, slot0):
                    for j in range(2):
                        po = 64 * j
                        rec = misc.tile([65, 512], F32R, tag='rec')
                        with nc.allow_low_precision(reason='softmax recip f32r'):
                            nc.vector.reciprocal(rec[64:65, :], p_av[j][64:65, :])
                        # broadcast partition 64 -> 0..63 via a 1-deep matmul
                        # in PE array row group 64
                        p_rb = psrb.tile([64, 512], F32, tag='rb')
                        nc.tensor.matmul(p_rb[:], ones_t[64:65, :],
                                         rec[64:65, :], start=True, stop=True,
                                         tile_position=(64, 0))
                        rec_b = misc.tile([64, 512], F32, tag='recb')
                        nc.scalar.activation(rec_b[:], p_rb[:], AF.Identity)
                        o_tmp = misc.tile([64, 512], F32R, tag='otmp')
                        with nc.allow_low_precision(reason='attn out f32r'):
                            nc.vector.tensor_mul(o_tmp[:], p_av[j][0:64, :],
                                                 rec_b[:])
                        nc.gpsimd.dma_start(
                            o_sb[m][po:po + 64, sc * 512:(sc + 1) * 512],
                            o_tmp[:])

                for m in range(CT):
                    for sc in range(SC):
                        p_av = [psav.tile([65, 512], F32, tag=f'av{j}',
                                          name=f'av{j}') for j in range(2)]
                        for tt in range(UT):
                            p_sc = [pssc.tile([128, 512], F32, tag=f'sc{j}',
                                              name=f'sc{j}') for j in range(2)]
                            for j in range(2):
                                po = 64 * j
                                nc.tensor.matmul(
                                    p_sc[j][:],
                                    k_sb[m][po:po + 64, tt * 128:(tt + 1) * 128],
                                    q_sb[m][po:po + 64, sc * 512:(sc + 1) * 512],
                                    start=True, stop=True,
                                    tile_position=(po, 0))
                            for j in range(2):
                                e_t = epool.tile([128, 512], F32R, tag='e')
                                nc.scalar.activation(e_t[:], p_sc[j][:],
                                                     AF.Exp, scale=scale)
                                h = 2 * m + j
                                nc.tensor.matmul(
                                    p_av[j][:],
                                    vt_sb[tt][:, 65 * h:65 * h + 65],
                                    e_t[:], start=(tt == 0),
                                    stop=(tt == UT - 1))
                        finish_heads(m, sc, p_av, 0)

            # out projection + residual
            x2_sb = []
            with tc.tile_pool(name='ps_proj2', bufs=2, space='PSUM') as ppp:
                for ot in range(CT):
                    t = new_resid(ot)
                    for sc in range(SC):
                        p = ppp.tile([128, 512], F32, tag='pp')
                        for ct in range(CT):
                            nc.tensor.matmul(
                                p[:], wo_sb[ct][:, ot * 128:(ot + 1) * 128],
                                o_sb[ct][:, sc * 512:(sc + 1) * 512],
                                start=(ct == 0), stop=(ct == CT - 1))
                        nc.vector.scalar_tensor_tensor(
                            t[:, sc * 512:(sc + 1) * 512], p[:],
                            bo_t[:, ot:ot + 1],
                            x1_sb[ot][:, sc * 512:(sc + 1) * 512],
                            op0=ALU.add, op1=ALU.add)
                    x2_sb.append(t)
        wqkv.release()

        # ================= Conv module =================
        # u_pad: CP fp8 pair-tiles [128, 2*UPW]; half j holds channels
        # (2cp+j)*128... as conv input columns  0..14 zero | u | zero ..UPW
        with tc.tile_pool(name='upad', bufs=1) as up_pool:
            u_pad = [up_pool.tile([128, 2 * UPW], FP8, tag=f'u{i}', name=f'u{i}')
                     for i in range(CP)]
            zp = din['zpad'].ap()
            for i in range(CP):
                for half in range(2):
                    base = half * UPW
                    for off, n in ((0, PAD), (PAD + S, UPW - PAD - S)):
                        nc.sync.dma_start(
                            u_pad[i][:, base + off:base + off + n],
                            bass.AP(tensor=zp.tensor, offset=0,
                                    ap=[[0, 128], [1, n]]))
            # pw1 + GLU (writes fp8 directly)
            with tc.tile_pool(name='wpw1', bufs=1) as wp1, \
                 tc.tile_pool(name='sig', bufs=2) as sigp, \
                 tc.tile_pool(name='ps_pw1', bufs=4, space='PSUM') as ps1:
                pw1_sb = []
                for i in range(CT):
                    t = wp1.tile([128, 2 * CONV_INNER], BF16, tag=f'pw1_{i}')
                    nc.sync.dma_start(t[:], din['w_pw1'].ap()[i * 128:(i + 1) * 128, :])
                    pw1_sb.append(t)
                bpw1_t = _bias_tile(nc, btp, din['b_pw1'], 2 * UT)
                x2b = cast_bf16(x2_sb)
                for ut in range(UT):
                    cp, half = divmod(ut, 2)
                    for sc in range(SC):
                        p_a = ps1.tile([128, 512], F32, tag='pp')
                        p_g = ps1.tile([128, 512], F32, tag='pp')
                        for ct in range(CT):
                            nc.tensor.matmul(
                                p_a[:], pw1_sb[ct][:, ut * 128:(ut + 1) * 128],
                                x2b[ct][:, sc * 512:(sc + 1) * 512],
                                start=(ct == 0), stop=(ct == CT - 1))
                        for ct in range(CT):
                            nc.tensor.matmul(
                                p_g[:], pw1_sb[ct][:, CONV_INNER + ut * 128:CONV_INNER + (ut + 1) * 128],
                                x2b[ct][:, sc * 512:(sc + 1) * 512],
                                start=(ct == 0), stop=(ct == CT - 1))
                        sig = sigp.tile([128, 512], F32, tag='sig')
                        nc.scalar.activation(sig[:], p_g[:], AF.Sigmoid,
                                             bias=bpw1_t[:, UT + ut:UT + ut + 1])
                        nc.vector.scalar_tensor_tensor(
                            u_pad[cp][:, half * UPW + PAD + sc * 512:
                                       half * UPW + PAD + (sc + 1) * 512],
                            p_a[:], bpw1_t[:, ut:ut + 1], sig[:],
                            op0=ALU.add, op1=ALU.mult)

            # dense conv1d over seq (K=31) as fp8 DoubleRow matmuls + silu
            h_sb = [hidb_tile(i) for i in range(UT)]
            with tc.tile_pool(name='wdc', bufs=2) as wdc, \
                 tc.tile_pool(name='ps_dc', bufs=4, space='PSUM') as psd:
                bdc_t = _bias_tile(nc, btp, din['b_dc'], UT)
                for ot in range(UT):
                    ps_c = [psd.tile([128, 512], F32, tag='cv', name=f'cv{_sc}')
                            for _sc in range(SC)]
                    for cp in range(CP):
                        wt = wdc.tile([128, K * 256], FP8, tag='dw')
                        nc.sync.dma_start(wt[:], din['w_dc'].ap()[ot, cp])
                        u3 = u_pad[cp][:].rearrange('p (two s) -> p two s', two=2)
                        for k in range(K):
                            w3 = wt[:, k * 256:(k + 1) * 256].rearrange(
                                'p (two f) -> p two f', two=2)
                            for sc in range(SC):
                                nc.tensor.matmul(
                                    ps_c[sc][:], w3,
                                    u3[:, :, k + sc * 512:k + sc * 512 + 512],
                                    start=(cp == 0 and k == 0),
                                    stop=(cp == CP - 1 and k == K - 1),
                                    perf_mode=DR)
                    for sc in range(SC):
                        with nc.allow_low_precision(reason='conv h bf16'):
                            nc.scalar.activation(
                                h_sb[ot][:, sc * 512:(sc + 1) * 512],
                                ps_c[sc][:], AF.Silu,
                                bias=bdc_t[:, ot:ot + 1], scale=PSUM_SCALE)

        # pw2 + residual
        x3_sb = []
        with tc.tile_pool(name='wpw2', bufs=1) as wp2, \
             tc.tile_pool(name='ps_pw2', bufs=4, space='PSUM') as ps2:
            pw2_sb = []
            for i in range(UT):
                t = wp2.tile([128, DIM], BF16, tag=f'pw2_{i}')
                nc.sync.dma_start(t[:], din['w_pw2'].ap()[i * 128:(i + 1) * 128, :])
                pw2_sb.append(t)
            bpw2_t = _bias_tile(nc, btp, din['b_pw2'], CT)
            for ot in range(CT):
                t = new_resid(ot)
                for sc in range(SC):
                    p = ps2.tile([128, 512], F32, tag='pp')
                    for ct in range(UT):
                        nc.tensor.matmul(
                            p[:], pw2_sb[ct][:, ot * 128:(ot + 1) * 128],
                            h_sb[ct][:, sc * 512:(sc + 1) * 512],
                            start=(ct == 0), stop=(ct == UT - 1))
                    nc.vector.scalar_tensor_tensor(
                        t[:, sc * 512:(sc + 1) * 512], p[:],
                        bpw2_t[:, ot:ot + 1],
                        x2_sb[ot][:, sc * 512:(sc + 1) * 512],
                        op0=ALU.add, op1=ALU.add)
                x3_sb.append(t)

        # ================= FF2 =================
        x4_sb = ff_block(x3_sb, din['w_ff2_1'], din['b_ff2_1'],
                         din['w_ff2_2'], din['b_ff2_2'], 'ff2')

        # ================= final affine + store =================
        with tc.tile_pool(name='fin', bufs=2) as fp:
            fing_t = _bias_tile(nc, btp, din['fin_g'], CT)
            finb_t = _bias_tile(nc, btp, din['fin_b'], CT)
            for ot in range(CT):
                o_t = fp.tile([128, S], F32, tag='out')
                nc.vector.tensor_scalar(
                    o_t[:], x4_sb[ot][:], fing_t[:, ot:ot + 1],
                    finb_t[:, ot:ot + 1], op0=ALU.mult, op1=ALU.add)
                nc.sync.dma_start(out_d.ap()[ot * 128:(ot + 1) * 128, :], o_t[:])


_prog_cache = {}


def _get_program():
    if 'nc' not in _prog_cache:
        _prog_cache['nc'] = build_program()
    return _prog_cache['nc']


def kernel(**inputs):
    inputs = {k: np.asarray(v, dtype=np.float32) for k, v in inputs.items()}
    w = _host_prep(inputs)
    nc = _get_program()
    x = inputs['x'][..., 0]  # [B, DIM, S]
    in_maps = [dict(w, x=np.ascontiguousarray(x[b])) for b in range(N_CORES)]
    res = run_bass_kernel_spmd(nc, in_maps, core_ids=list(range(N_CORES)))
    out = np.stack([res.results[b]['out'] for b in range(N_CORES)])
    return out[..., None].astype(np.float32)


# revision 58
# speedup vs baseline: 1.0019x; 1.0019x over previous
"""Conformer block kernel for 8 Trainium2 NeuronCores.

Sharding: pure data-parallel over batch (B=8 -> 1 batch element per core,
zero collectives). All weights are replicated; BatchNorm affines and scalar
multipliers are folded into the adjacent pointwise-conv weights on the host.

Matmul precision: everything runs as float32r (full PE rate) EXCEPT the
dense depthwise-ish conv (1024x1024xK=31 contraction, ~85% of all FLOPs),
which runs as fp8(e4m3) with perf_mode=DoubleRow (2 contraction rows per
PE pass). Weights are pre-scaled by 2^6 on the host (and the GLU 'a' path
by 2^2) so fp8 values sit in the normal range; the product scale 2^-8 is
folded into the PSUM->SBUF Silu activation.

Other PE savings vs the plain version: the softmax denominator rides as a
65th "ones" column of V (no separate ones-matmul), and the S=64-contraction
score matmuls for a head pair run concurrently in disjoint PE row groups
via tile_position.
"""
import sys

sys.path.insert(0, '/opt/trn_rl_repo')

import numpy as np
import ml_dtypes
import concourse.bass as bass
import concourse.tile as tile
from concourse import bacc, mybir
from concourse.bass_utils import run_bass_kernel_spmd

F32 = mybir.dt.float32
F32R = mybir.dt.float32r
BF16 = mybir.dt.bfloat16
FP8 = mybir.dt.float8e4
AF = mybir.ActivationFunctionType
ALU = mybir.AluOpType
DR = mybir.MatmulPerfMode.DoubleRow
E4NP = ml_dtypes.float8_e4m3fn

B, DIM, S = 8, 512, 1024
H, DH = 8, 64
FF_INNER = 1024
CONV_INNER = 1024
K = 31
PAD = (K - 1) // 2
N_CORES = 8

CT = DIM // 128          # 4  channel tiles of the 512-dim stream
UT = CONV_INNER // 128   # 8  tiles of 1024-wide inner dims
SC = S // 512            # 2  free-dim chunks of 512
CP = UT // 2             # 4  fp8 pair-tiles of the conv inner dim
UPW = PAD + S + 17       # 1056: padded conv input row (15 left, 17 right)

W_SCALE = 64.0           # dconv weight pre-scale (keeps fp8 out of subnormals)
U_SCALE = 1.0            # GLU 'a'-path pre-scale (no measurable gain beyond 1)
PSUM_SCALE = 1.0 / (W_SCALE * U_SCALE)


def _host_prep(i):
    """Fold affines/scalars into weights; pre-transpose for lhsT layout."""
    f = np.float32
    w = {}

    BF = ml_dtypes.bfloat16

    def fold(wmat, g, b, bout, dt=f):
        # y = wmat @ (g*x + b) + bout  ->  W' = wmat * g[None, :],
        # b' = wmat @ b + bout ; return transposed W' [in, out]
        wp = (wmat * g[None, :]).astype(f)
        bp = (wmat @ b + bout).astype(f)
        return np.ascontiguousarray(wp.T).astype(dt), bp

    w['w_ff1_1'], w['b_ff1_1'] = fold(i['ff1_w1'], i['ff1_g'], i['ff1_b'], i['ff1_b1'], BF)
    w['w_ff1_2'] = np.ascontiguousarray((0.5 * i['ff1_w2']).T.astype(f)).astype(BF)
    w['b_ff1_2'] = (0.5 * i['ff1_b2']).astype(f)

    w['w_q'], w['b_q'] = fold(i['wq'], i['attn_g'], i['attn_b'], i['bq'])
    w['w_k'], w['b_k'] = fold(i['wk'], i['attn_g'], i['attn_b'], i['bk'])
    w['w_v'], w['b_v'] = fold(i['wv'], i['attn_g'], i['attn_b'], i['bv'])
    w['w_o'] = np.ascontiguousarray(i['wo'].T.astype(f))
    w['b_o'] = i['bo'].astype(f)

    # pw1: scale the GLU 'a' half (first CONV_INNER out channels) by U_SCALE
    # so the fp8 conv input u = U_SCALE * a * sigmoid(gate).
    pw1_w = i['pw1_w'].copy()
    pw1_b = i['pw1_b'].copy()
    pw1_w[:CONV_INNER] *= U_SCALE
    pw1_b[:CONV_INNER] *= U_SCALE
    w['w_pw1'], w['b_pw1'] = fold(pw1_w, i['conv_g'], i['conv_b'], pw1_b, BF)

    # dconv: fold cbn_g into weights, pre-scale by W_SCALE, quantize to fp8.
    # Layout per (ot, cp): a [128(c_lo), 31*2*128] lhsT strip where the free
    # offset is k*256 + c_hi*128 + co — each k-slice is a DoubleRow
    # [128, 2, 128] stationary operand pairing channels c_hi=0/1.
    dw = (i['dconv_w'][:, :, 0, :] * i['cbn_g'][:, None, None]).astype(f)  # [o,c,k]
    dw = np.clip(dw * W_SCALE, -240.0, 240.0)
    a = dw.reshape(UT, 128, CP, 2, 128, K)         # [ot, co, cp, chi, clo, k]
    a = a.transpose(0, 2, 4, 5, 3, 1)              # [ot, cp, clo, k, chi, co]
    w['w_dc'] = np.ascontiguousarray(a.reshape(UT, CP, 128, K * 256)).astype(E4NP)
    w['b_dc'] = (i['cbn_g'] * i['dconv_b'] + i['cbn_b']).astype(f)
    w['w_pw2'] = np.ascontiguousarray(i['pw2_w'].T.astype(f)).astype(BF)
    w['b_pw2'] = i['pw2_b'].astype(f)

    w['w_ff2_1'], w['b_ff2_1'] = fold(i['ff2_w1'], i['ff2_g'], i['ff2_b'], i['ff2_b1'], BF)
    w['w_ff2_2'] = np.ascontiguousarray((0.5 * i['ff2_w2']).T.astype(f)).astype(BF)
    w['b_ff2_2'] = (0.5 * i['ff2_b2']).astype(f)

    w['fin_g'] = i['fin_g'].astype(f)
    w['fin_b'] = i['fin_b'].astype(f)
    w['zpad'] = np.zeros(17, E4NP)
    w['onesv'] = np.ones(64, f)
    return w


def _bias_tile(nc, sb, dram_vec, n):
    """Load a [n*128] DRAM vector as a [128, n] SBUF tile (col t = tile t)."""
    t = sb.tile([128, n], F32, tag=f'bias_{dram_vec.name}', name=f'b_{dram_vec.name}')
    nc.sync.dma_start(t[:], dram_vec.ap().rearrange('(t p) -> p t', p=128))
    return t


def _bcast_tile(nc, sb, dram_vec, n, tag):
    """Broadcast a [n] DRAM vector across 128 partitions -> [128, n] f32r."""
    t = sb.tile([128, n], F32R, tag=tag, name=tag)
    v = dram_vec.ap()
    nc.sync.dma_start(
        t[:], bass.AP(tensor=v.tensor, offset=0, ap=[[0, 128], [1, n]]))
    return t


def build_program():
    nc = bacc.Bacc('TRN2', target_bir_lowering=False, debug=False)
    dt_in = {}

    def din(name, shape, dt=F32R):
        dt_in[name] = nc.dram_tensor(name, shape, dt, kind='ExternalInput')
        return dt_in[name]

    din('x', [DIM, S])
    din('w_ff1_1', [DIM, FF_INNER], BF16); din('b_ff1_1', [FF_INNER], F32)
    din('w_ff1_2', [FF_INNER, DIM], BF16); din('b_ff1_2', [DIM], F32)
    din('w_q', [DIM, DIM]); din('b_q', [DIM], F32)
    din('w_k', [DIM, DIM]); din('b_k', [DIM], F32)
    din('w_v', [DIM, DIM]); din('b_v', [DIM])
    din('w_o', [DIM, DIM]); din('b_o', [DIM], F32)
    din('w_pw1', [DIM, 2 * CONV_INNER], BF16); din('b_pw1', [2 * CONV_INNER], F32)
    din('w_dc', [UT, CP, 128, K * 256], FP8)
    din('b_dc', [CONV_INNER], F32)
    din('w_pw2', [CONV_INNER, DIM], BF16); din('b_pw2', [DIM], F32)
    din('w_ff2_1', [DIM, FF_INNER], BF16); din('b_ff2_1', [FF_INNER], F32)
    din('w_ff2_2', [FF_INNER, DIM], BF16); din('b_ff2_2', [DIM], F32)
    din('fin_g', [DIM], F32); din('fin_b', [DIM], F32)
    din('zpad', [17], FP8)
    din('onesv', [64])
    out_d = nc.dram_tensor('out', [DIM, S], F32, kind='ExternalOutput')

    with tile.TileContext(nc, pool_alloc_mode='queue') as tc:
        _emit(nc, tc, dt_in, out_d)
    nc.compile()
    return nc


def _emit(nc, tc, din, out_d):
    from contextlib import ExitStack
    ctx = ExitStack()
    with ctx:
        # ---- persistent pools -------------------------------------------
        resid = ctx.enter_context(tc.tile_pool(name='resid', bufs=2))
        hid = ctx.enter_context(tc.tile_pool(name='hid', bufs=1))
        btp = ctx.enter_context(tc.tile_pool(name='biases', bufs=1))

        def new_resid(i):
            return resid.tile([128, S], F32R, tag=f'r{i}', name=f'r{i}')

        def hid_tile(i):
            return hid.tile([128, S], F32R, tag=f'h{i}', name=f'h{i}')

        def hidb_tile(i):
            return hid.tile([128, S], BF16, tag=f'hb{i}', name=f'hb{i}')

        xcast = ctx.enter_context(tc.tile_pool(name='xcast', bufs=2))

        def cast_bf16(x_in):
            out = []
            for i, t in enumerate(x_in):
                b = xcast.tile([128, S], BF16, tag=f'xb{i}', name=f'xb{i}')
                with nc.allow_low_precision(reason='bf16 matmul input'):
                    nc.vector.tensor_scalar_add(b[:], t[:], 0.0)
                out.append(b)
            return out

        # load x
        x_sb = []
        for i in range(CT):
            t = new_resid(i)
            nc.sync.dma_start(t[:], din['x'].ap()[i * 128:(i + 1) * 128, :])
            x_sb.append(t)

        def load_ff_weights(wp, w1d, w2d):
            w1_sb = []
            for i in range(CT):
                t = wp.tile([128, FF_INNER], BF16, tag=f'w1_{i}')
                nc.sync.dma_start(t[:], w1d.ap()[i * 128:(i + 1) * 128, :])
                w1_sb.append(t)
            w2_sb = []
            for i in range(UT):
                t = wp.tile([128, DIM], BF16, tag=f'w2_{i}')
                nc.sync.dma_start(t[:], w2d.ap()[i * 128:(i + 1) * 128, :])
                w2_sb.append(t)
            return w1_sb, w2_sb

        # Pool alloc order fixes the (LIFO) release order: wqkv is released
        # right after the attention block, wff1 right after FF1. The DMA
        # issue order below is what controls arrival: x, then FF1 weights
        # (the first matmul waits on them), then the attention weights.
        wattn = ctx.enter_context(tc.tile_pool(name='wattn', bufs=1))
        wqkv = tc.alloc_tile_pool(name='wqkv', bufs=1)
        wff1 = tc.alloc_tile_pool(name='wff1', bufs=1)

        ff1_w1_sb, ff1_w2_sb = load_ff_weights(wff1, din['w_ff1_1'],
                                               din['w_ff1_2'])
        wq_sb, wk_sb, wv_sb, wo_sb = [], [], [], []
        for nm, lst in (('w_q', wq_sb), ('w_k', wk_sb), ('w_v', wv_sb),
                        ('w_o', wo_sb)):
            pool = wattn if nm == 'w_o' else wqkv
            for i in range(CT):
                t = pool.tile([128, DIM], F32R, tag=f'{nm}_{i}', name=f'{nm}_{i}')
                nc.sync.dma_start(t[:], din[nm].ap()[i * 128:(i + 1) * 128, :])
                lst.append(t)
        bv_bc = _bcast_tile(nc, wqkv, din['b_v'], DIM, 'bv_bc')
        # ones row AT partition 64 (stationary operand of the reciprocal
        # partition-broadcast matmul, which contracts over array row 64)
        ones_t = wattn.tile([65, 64], F32R, tag='ones_t', name='ones_t')
        ov = din['onesv'].ap()
        nc.sync.dma_start(
            ones_t[64:65, :],
            bass.AP(tensor=ov.tensor, offset=0, ap=[[0, 1], [1, 64]]))
        bq_t = _bias_tile(nc, btp, din['b_q'], CT)
        bk_t = _bias_tile(nc, btp, din['b_k'], CT)
        bo_t = _bias_tile(nc, btp, din['b_o'], CT)

        # ---- generic FF macro -------------------------------------------
        def ff_block(x_in, w1d, b1d, w2d, b2d, nm, preloaded=None):
            with tc.tile_pool(name=f'w{nm}', bufs=1) as wp, \
                 tc.tile_pool(name=f'ps{nm}', bufs=4, space='PSUM') as ps:
                if preloaded is None:
                    w1_sb, w2_sb = load_ff_weights(wp, w1d, w2d)
                else:
                    w1_sb, w2_sb = preloaded
                b1_t = _bias_tile(nc, btp, b1d, UT)
                b2_t = _bias_tile(nc, btp, b2d, CT)

                xb = cast_bf16(x_in)
                h_sb = [hidb_tile(i) for i in range(UT)]
                for ot in range(UT):
                    for sc in range(SC):
                        p = ps.tile([128, 512], F32, tag='pp')
                        for ct in range(CT):
                            nc.tensor.matmul(
                                p[:], w1_sb[ct][:, ot * 128:(ot + 1) * 128],
                                xb[ct][:, sc * 512:(sc + 1) * 512],
                                start=(ct == 0), stop=(ct == CT - 1))
                        with nc.allow_low_precision(reason='ff h bf16'):
                            nc.scalar.activation(
                                h_sb[ot][:, sc * 512:(sc + 1) * 512], p[:],
                                AF.Silu, bias=b1_t[:, ot:ot + 1])
                x_out = []
                for ot in range(CT):
                    t = new_resid(ot)
                    for sc in range(SC):
                        p = ps.tile([128, 512], F32, tag='pp')
                        for ct in range(UT):
                            nc.tensor.matmul(
                                p[:], w2_sb[ct][:, ot * 128:(ot + 1) * 128],
                                h_sb[ct][:, sc * 512:(sc + 1) * 512],
                                start=(ct == 0), stop=(ct == UT - 1))
                        nc.vector.scalar_tensor_tensor(
                            t[:, sc * 512:(sc + 1) * 512], p[:],
                            b2_t[:, ot:ot + 1],
                            x_in[ot][:, sc * 512:(sc + 1) * 512],
                            op0=ALU.add, op1=ALU.add)
                    x_out.append(t)
                return x_out

        # ================= FF1 =================
        x1_sb = ff_block(x_sb, din['w_ff1_1'], din['b_ff1_1'],
                         din['w_ff1_2'], din['b_ff1_2'], 'ff1',
                         preloaded=(ff1_w1_sb, ff1_w2_sb))
        wff1.release()

        # ================= Attention =================
        with tc.tile_pool(name='attn_sb', bufs=1) as asb, \
             tc.tile_pool(name='attn_e', bufs=4) as epool, \
             tc.tile_pool(name='attn_misc', bufs=2) as misc:
            # Q, K projections (standard layout), reusing hid slots
            q_sb = [hid_tile(i) for i in range(CT)]
            k_sb = [hid_tile(CT + i) for i in range(CT)]
            vt_sb = []
            with tc.tile_pool(name='ps_proj1', bufs=2, space='PSUM') as ppp:
                for dst, w_sb, b_t in ((q_sb, wq_sb, bq_t), (k_sb, wk_sb, bk_t)):
                    for ot in range(CT):
                        for sc in range(SC):
                            p = ppp.tile([128, 512], F32, tag='pp')
                            for ct in range(CT):
                                nc.tensor.matmul(
                                    p[:], w_sb[ct][:, ot * 128:(ot + 1) * 128],
                                    x1_sb[ct][:, sc * 512:(sc + 1) * 512],
                                    start=(ct == 0), stop=(ct == CT - 1))
                            nc.scalar.activation(
                                dst[ot][:, sc * 512:(sc + 1) * 512], p[:],
                                AF.Identity, bias=b_t[:, ot:ot + 1])
                # V transposed, with a 65th all-ones column per head so the AV
                # matmul also produces the softmax denominator in row 64.
                # vt[tt] is [128(t), 8*65] : head h at cols 65h..65h+64.
                for tt in range(UT):
                    p = ppp.tile([128, 512], F32, tag='pp')
                    for ct in range(CT):
                        nc.tensor.matmul(
                            p[:], x1_sb[ct][:, tt * 128:(tt + 1) * 128],
                            wv_sb[ct][:], start=(ct == 0), stop=(ct == CT - 1))
                    t = asb.tile([128, 8 * 65], F32R, tag=f'vt{tt}')
                    t3 = t[:].rearrange('p (h d) -> p h d', d=65)
                    nc.vector.tensor_tensor(
                        t3[:, :, 0:64],
                        p[:].rearrange('p (h d) -> p h d', d=64),
                        bv_bc[:].rearrange('p (h d) -> p h d', d=64),
                        op=ALU.add)
                    o8 = din['onesv'].ap()
                    nc.sync.dma_start(
                        t3[:, :, 64:65],
                        bass.AP(tensor=o8.tensor, offset=0,
                                ap=[[0, 128], [1, 8], [0, 1]]))
                    vt_sb.append(t)

            # Per-head attention, software-pipelined over stages (m, sc):
            # stage g issues scores+exp for g interleaved with the AV matmuls
            # of stage g-1, so the PE never idles long enough for the HAM
            # clock gate to re-throttle while the scalar engine chews exps.
            # Head pair (2m, 2m+1) lives at partitions 0-63 / 64-127 of slab
            # m; its two score matmuls run concurrently in disjoint PE row
            # groups via tile_position.
            o_sb = [asb.tile([128, S], F32R, tag=f'o{i}', name=f'o{i}')
                    for i in range(CT)]
            scale = float(DH) ** -0.5

            with tc.tile_pool(name='ps_sc', bufs=2, space='PSUM') as pssc, \
                 tc.tile_pool(name='ps_av', bufs=1, space='PSUM') as psav, \
                 tc.tile_pool(name='ps_rb', bufs=1, space='PSUM') as psrb, \
                 tc.tile_pool(name='ps_dum', bufs=1, space='PSUM') as psdum:

                def keep_warm():
                    # 32 back-to-back matmuls on resident operands into a
                    # dead PSUM bank: a guaranteed >2x3.4us contiguous PE
                    # burst that re-arms the HAM clock gate (the fine-grained
                    # score/exp ping-pong never does), so the whole attention
                    # phase runs at 2.4 GHz instead of 1.2.
                    pd = psdum.tile([128, 512], F32, tag='dum')
                    for r in range(32):
                        nc.tensor.matmul(
                            pd[:], wo_sb[0][:, 0:128], x1_sb[0][:, 0:512],
                            start=(r == 0), stop=(r == 31))

                def finish_heads(m, sc, p_av, slot0):
                    for j in range(2):
                        po = 64 * j
                        rec = misc.tile([65, 512], F32R, tag='rec')
                        with nc.allow_low_precision(reason='softmax recip f32r'):
                            nc.vector.reciprocal(rec[64:65, :], p_av[j][64:65, :])
                        # broadcast partition 64 -> 0..63 via a 1-deep matmul
                        # in PE array row group 64
                        p_rb = psrb.tile([64, 512], F32, tag='rb')
                        nc.tensor.matmul(p_rb[:], ones_t[64:65, :],
                                         rec[64:65, :], start=True, stop=True,
                                         tile_position=(64, 0))
                        rec_b = misc.tile([64, 512], F32, tag='recb')
                        nc.scalar.activation(rec_b[:], p_rb[:], AF.Identity)
                        o_tmp = misc.tile([64, 512], F32R, tag='otmp')
                        with nc.allow_low_precision(reason='attn out f32r'):
                            nc.vector.tensor_mul(o_tmp[:], p_av[j][0:64, :],
                                                 rec_b[:])
                        nc.gpsimd.dma_start(
                            o_sb[m][po:po + 64, sc * 512:(sc + 1) * 512],
                            o_tmp[:])

                for m in range(CT):
                    for sc in range(SC):
                        p_av = [psav.tile([65, 512], F32, tag=f'av{j}',
                                          name=f'av{j}') for j in range(2)]
                        for tt in range(UT):
                            p_sc = [pssc.tile([128, 512], F32, tag=f'sc{j}',
                                              name=f'sc{j}') for j in range(2)]
                            for j in range(2):
                                po = 64 * j
                                nc.tensor.matmul(
                                    p_sc[j][:],
                                    k_sb[m][po:po + 64, tt * 128:(tt + 1) * 128],
                                    q_sb[m][po:po + 64, sc * 512:(sc + 1) * 512],
                                    start=True, stop=True,
                                    tile_position=(po, 0))
                            for j in range(2):
                                e_t = epool.tile([128, 512], F32R, tag='e')
                                nc.scalar.activation(e_t[:], p_sc[j][:],
                                                     AF.Exp, scale=scale)
                                h = 2 * m + j
                                nc.tensor.matmul(
                                    p_av[j][:],
                                    vt_sb[tt][:, 65 * h:65 * h + 65],
                                    e_t[:], start=(tt == 0),
                                    stop=(tt == UT - 1))
                            if tt == 3:
                                keep_warm()
                        finish_heads(m, sc, p_av, 0)

            # out projection + residual
            x2_sb = []
            with tc.tile_pool(name='ps_proj2', bufs=2, space='PSUM') as ppp:
                for ot in range(CT):
                    t = new_resid(ot)
                    for sc in range(SC):
                        p = ppp.tile([128, 512], F32, tag='pp')
                        for ct in range(CT):
                            nc.tensor.matmul(
                                p[:], wo_sb[ct][:, ot * 128:(ot + 1) * 128],
                                o_sb[ct][:, sc * 512:(sc + 1) * 512],
                                start=(ct == 0), stop=(ct == CT - 1))
                        nc.vector.scalar_tensor_tensor(
                            t[:, sc * 512:(sc + 1) * 512], p[:],
                            bo_t[:, ot:ot + 1],
                            x1_sb[ot][:, sc * 512:(sc + 1) * 512],
                            op0=ALU.add, op1=ALU.add)
                    x2_sb.append(t)
        wqkv.release()

        # ================= Conv module =================
        # u_pad: CP fp8 pair-tiles [128, 2*UPW]; half j holds channels
        # (2cp+j)*128... as conv input columns  0..14 zero | u | zero ..UPW
        with tc.tile_pool(name='upad', bufs=1) as up_pool:
            u_pad = [up_pool.tile([128, 2 * UPW], FP8, tag=f'u{i}', name=f'u{i}')
                     for i in range(CP)]
            zp = din['zpad'].ap()
            for i in range(CP):
                for half in range(2):
                    base = half * UPW
                    for off, n in ((0, PAD), (PAD + S, UPW - PAD - S)):
                        nc.sync.dma_start(
                            u_pad[i][:, base + off:base + off + n],
                            bass.AP(tensor=zp.tensor, offset=0,
                                    ap=[[0, 128], [1, n]]))
            # pw1 + GLU (writes fp8 directly)
            with tc.tile_pool(name='wpw1', bufs=1) as wp1, \
                 tc.tile_pool(name='sig', bufs=2) as sigp, \
                 tc.tile_pool(name='ps_pw1', bufs=4, space='PSUM') as ps1:
                pw1_sb = []
                for i in range(CT):
                    t = wp1.tile([128, 2 * CONV_INNER], BF16, tag=f'pw1_{i}')
                    nc.sync.dma_start(t[:], din['w_pw1'].ap()[i * 128:(i + 1) * 128, :])
                    pw1_sb.append(t)
                bpw1_t = _bias_tile(nc, btp, din['b_pw1'], 2 * UT)
                x2b = cast_bf16(x2_sb)
                for ut in range(UT):
                    cp, half = divmod(ut, 2)
                    for sc in range(SC):
                        p_a = ps1.tile([128, 512], F32, tag='pp')
                        p_g = ps1.tile([128, 512], F32, tag='pp')
                        for ct in range(CT):
                            nc.tensor.matmul(
                                p_a[:], pw1_sb[ct][:, ut * 128:(ut + 1) * 128],
                                x2b[ct][:, sc * 512:(sc + 1) * 512],
                                start=(ct == 0), stop=(ct == CT - 1))
                        for ct in range(CT):
                            nc.tensor.matmul(
                                p_g[:], pw1_sb[ct][:, CONV_INNER + ut * 128:CONV_INNER + (ut + 1) * 128],
                                x2b[ct][:, sc * 512:(sc + 1) * 512],
                                start=(ct == 0), stop=(ct == CT - 1))
                        sig = sigp.tile([128, 512], F32, tag='sig')
                        nc.scalar.activation(sig[:], p_g[:], AF.Sigmoid,
                                             bias=bpw1_t[:, UT + ut:UT + ut + 1])
                        nc.vector.scalar_tensor_tensor(
                            u_pad[cp][:, half * UPW + PAD + sc * 512:
                                       half * UPW + PAD + (sc + 1) * 512],
                            p_a[:], bpw1_t[:, ut:ut + 1], sig[:],
                            op0=ALU.add, op1=ALU.mult)

            # dense conv1d over seq (K=31) as fp8 DoubleRow matmuls + silu
            h_sb = [hidb_tile(i) for i in range(UT)]
            with tc.tile_pool(name='wdc', bufs=2) as wdc, \
                 tc.tile_pool(name='ps_dc', bufs=4, space='PSUM') as psd:
                bdc_t = _bias_tile(nc, btp, din['b_dc'], UT)
                for ot in range(UT):
                    ps_c = [psd.tile([128, 512], F32, tag='cv', name=f'cv{_sc}')
                            for _sc in range(SC)]
                    for cp in range(CP):
                        wt = wdc.tile([128, K * 256], FP8, tag='dw')
                        nc.sync.dma_start(wt[:], din['w_dc'].ap()[ot, cp])
                        u3 = u_pad[cp][:].rearrange('p (two s) -> p two s', two=2)
                        for k in range(K):
                            w3 = wt[:, k * 256:(k + 1) * 256].rearrange(
                                'p (two f) -> p two f', two=2)
                            for sc in range(SC):
                                nc.tensor.matmul(
                                    ps_c[sc][:], w3,
                                    u3[:, :, k + sc * 512:k + sc * 512 + 512],
                                    start=(cp == 0 and k == 0),
                                    stop=(cp == CP - 1 and k == K - 1),
                                    perf_mode=DR)
                    for sc in range(SC):
                        with nc.allow_low_precision(reason='conv h bf16'):
                            nc.scalar.activation(
                                h_sb[ot][:, sc * 512:(sc + 1) * 512],
                                ps_c[sc][:], AF.Silu,
                                bias=bdc_t[:, ot:ot + 1], scale=PSUM_SCALE)

        # pw2 + residual
        x3_sb = []
        with tc.tile_pool(name='wpw2', bufs=1) as wp2, \
             tc.tile_pool(name='ps_pw2', bufs=4, space='PSUM') as ps2:
            pw2_sb = []
            for i in range(UT):
                t = wp2.tile([128, DIM], BF16, tag=f'pw2_{i}')
                nc.sync.dma_start(t[:], din['w_pw2'].ap()[i * 128:(i + 1) * 128, :])
                pw2_sb.append(t)
            bpw2_t = _bias_tile(nc, btp, din['b_pw2'], CT)
            for ot in range(CT):
                t = new_resid(ot)
                for sc in range(SC):
                    p = ps2.tile([128, 512], F32, tag='pp')
                    for ct in range(UT):
                        nc.tensor.matmul(
                            p[:], pw2_sb[ct][:, ot * 128:(ot + 1) * 128],
                            h_sb[ct][:, sc * 512:(sc + 1) * 512],
                            start=(ct == 0), stop=(ct == UT - 1))
                    nc.vector.scalar_tensor_tensor(
                        t[:, sc * 512:(sc + 1) * 512], p[:],
                        bpw2_t[:, ot:ot + 1],
                        x2_sb[ot][:, sc * 512:(sc + 1) * 512],
                        op0=ALU.add, op1=ALU.add)
                x3_sb.append(t)

        # ================= FF2 =================
        x4_sb = ff_block(x3_sb, din['w_ff2_1'], din['b_ff2_1'],
                         din['w_ff2_2'], din['b_ff2_2'], 'ff2')

        # ================= final affine + store =================
        with tc.tile_pool(name='fin', bufs=2) as fp:
            fing_t = _bias_tile(nc, btp, din['fin_g'], CT)
            finb_t = _bias_tile(nc, btp, din['fin_b'], CT)
            for ot in range(CT):
                o_t = fp.tile([128, S], F32, tag='out')
                nc.vector.tensor_scalar(
                    o_t[:], x4_sb[ot][:], fing_t[:, ot:ot + 1],
                    finb_t[:, ot:ot + 1], op0=ALU.mult, op1=ALU.add)
                nc.sync.dma_start(out_d.ap()[ot * 128:(ot + 1) * 128, :], o_t[:])


_prog_cache = {}


def _get_program():
    if 'nc' not in _prog_cache:
        _prog_cache['nc'] = build_program()
    return _prog_cache['nc']


def kernel(**inputs):
    inputs = {k: np.asarray(v, dtype=np.float32) for k, v in inputs.items()}
    w = _host_prep(inputs)
    nc = _get_program()
    x = inputs['x'][..., 0]  # [B, DIM, S]
    in_maps = [dict(w, x=np.ascontiguousarray(x[b])) for b in range(N_CORES)]
    res = run_bass_kernel_spmd(nc, in_maps, core_ids=list(range(N_CORES)))
    out = np.stack([res.results[b]['out'] for b in range(N_CORES)])
    return out[..., None].astype(np.float32)
